# revision 31
# baseline (speedup 1.0000x reference)
"""Multi-head attention (B=4, S=2048, D=1024, H=16) on 8 Trainium2 NeuronCores.

Sharding: core c handles batch b = c//2 and query-row half c%2 (1024 query
rows). Each core computes K/V projections for its batch's full 2048 kv rows
(duplicated across the pair), attention for its 1024 query rows over all 16
heads, and the output projection for its rows. Output is a pure concatenation
across cores — no collectives.

Device algorithm (per core, all matmuls bf16 with fp32 PSUM accumulation):
  QT[e,s]  = (WqT.T @ xqT) * 1/8 + bq/8          (e on partitions)
  KT[e,s]  =  WkT.T @ xkT                        (bk dropped: softmax-invariant)
  V[s,e]   =  xvT.T @ WvT                        (bv folded into output bias)
  per head h, q-chunk:
    scoresT[k,q] = KT_h.T @ QT_h      (K=64; head parity lives on partition
                                       halves -> row-group-packed matmul pairs)
    E = exp(scoresT)                  (ScalarE, PSUM->SBUF bf16 — the pacing
                                       engine; everything else hides under it)
    [y_h; rowsum] = [V_h | 1].T @ E   (M=65, ones col gives rowsum)
    yT_h = y_h * (1/rowsum)           (PE K=1 matmul replicates 1/rowsum
                                       across partitions; DVE multiply)
  out[q,e] = yT.T @ WoT + (bo + Wo@bv)

Schedule: Qproj -> Kproj(et 0,1) -> scores/exp(hp0,qc0) -> Vproj st-blocks
interleaved with attnV(hp0,qc0) (lag 1 st) and scores(hp0,qc1) -> per-hp
streams where attnV lags scores by one kt2 slot and Kproj(et+2) rides along.
"""

import numpy as np
import ml_dtypes

B, S, D = 4, 2048, 1024
H, DK = 16, 64
NCORES = 8
SQ = S // 2            # query rows per core
P = 128
DTILES = D // P        # 8
QCH = SQ // 512        # 2
KT_N = S // P          # 16
HP = H // 2            # 8 head pairs
CHUNK = 512

_CACHE = {}


def _build_nc():
    import concourse.mybir as mybir
    import concourse.tile as tile
    from concourse import bacc

    F32, BF16 = mybir.dt.float32, mybir.dt.bfloat16
    Exp = mybir.ActivationFunctionType.Exp

    nc = bacc.Bacc("TRN2", target_bir_lowering=False, debug=False,
                   num_devices=NCORES)

    xqT = nc.dram_tensor("xqT", [D, SQ], BF16, kind="ExternalInput").ap()
    xkT = nc.dram_tensor("xkT", [D, S], BF16, kind="ExternalInput").ap()
    xvT = nc.dram_tensor("xvT", [D, S], BF16, kind="ExternalInput").ap()
    wqT = nc.dram_tensor("wqT", [D, D], BF16, kind="ExternalInput").ap()
    wkT = nc.dram_tensor("wkT", [D, D], BF16, kind="ExternalInput").ap()
    wvT = nc.dram_tensor("wvT", [D, D], BF16, kind="ExternalInput").ap()
    woT = nc.dram_tensor("woT", [D, D], BF16, kind="ExternalInput").ap()
    bqs = nc.dram_tensor("bqs", [P, DTILES], F32, kind="ExternalInput").ap()
    bob = nc.dram_tensor("bob", [P, D], F32, kind="ExternalInput").ap()
    out = nc.dram_tensor("out", [SQ, D], F32, kind="ExternalOutput").ap()

    def pdt(ap):  # [D, N] dram -> [P, DTILES, N] partition-tiled view
        return ap.rearrange("(a p) n -> p a n", p=P)

    with tile.TileContext(nc) as tc:
        with (
            tc.tile_pool(name="wpool", bufs=2) as wpool,
            tc.tile_pool(name="xpool", bufs=2) as xpool,
            tc.tile_pool(name="cpool", bufs=1) as cpool,
            tc.tile_pool(name="epool", bufs=20) as epool,
            tc.tile_pool(name="ytpool", bufs=2) as ytpool,
            tc.tile_pool(name="npool", bufs=2) as npool,
            tc.tile_pool(name="opool", bufs=2) as opool,
            tc.tile_pool(name="psP", bufs=2, space="PSUM") as psP,
            tc.tile_pool(name="psS", bufs=2, space="PSUM") as psS,
            tc.tile_pool(name="psA", bufs=2, space="PSUM") as psA,
        ):
            # ---- constants / residents ----
            wq_s = wpool.tile([P, DTILES, D], BF16, tag="w", name="wq_s")
            for j in range(4):
                nc.sync.dma_start(wq_s[:, 2 * j:2 * j + 2, :],
                                  pdt(wqT)[:, 2 * j:2 * j + 2, :])
            bq_s = cpool.tile([P, DTILES], F32, name="bq_s")
            nc.sync.dma_start(bq_s[:], bqs[:])
            bob_s = cpool.tile([P, D], F32, name="bob_s")
            nc.sync.dma_start(bob_s[:], bob[:])
            ones_s = cpool.tile([1, DK], BF16, name="ones_s")
            nc.gpsimd.memset(ones_s[:], 1.0)

            qt_s = cpool.tile([P, DTILES, SQ], BF16, name="qt_s")
            kt_s = cpool.tile([P, DTILES, S], BF16, name="kt_s")
            va_s = cpool.tile([P, KT_N, H * (DK + 1)], BF16, name="va_s")
            nc.gpsimd.memset(va_s[:], 1.0)  # ones cols survive the V copies

            # ---- Q projection ----
            for qc in range(QCH):
                xq_c = xpool.tile([P, DTILES, CHUNK], BF16, tag="x",
                                  name="xq_c")
                for j4 in range(4):
                    nc.sync.dma_start(
                        xq_c[:, 2 * j4:2 * j4 + 2, :],
                        pdt(xqT[:, qc * CHUNK:(qc + 1) * CHUNK])
                        [:, 2 * j4:2 * j4 + 2, :])
                for et in range(DTILES):
                    psq = psP.tile([P, CHUNK], F32, tag="p", name="psq")
                    for dt in range(DTILES):
                        nc.tensor.matmul(
                            psq[:],
                            wq_s[:, dt, et * P:(et + 1) * P],
                            xq_c[:, dt, :],
                            start=(dt == 0), stop=(dt == DTILES - 1))
                    nc.vector.tensor_scalar(
                        qt_s[:, et, qc * CHUNK:(qc + 1) * CHUNK], psq[:],
                        0.125, bq_s[:, et:et + 1],
                        mybir.AluOpType.mult, mybir.AluOpType.add)

            wk_s = wpool.tile([P, DTILES, D], BF16, tag="w", name="wk_s")
            for j in range(4):
                nc.sync.dma_start(wk_s[:, 2 * j:2 * j + 2, :],
                                  pdt(wkT)[:, 2 * j:2 * j + 2, :])

            def kproj_sc(et, sc):
                xk_c = xpool.tile([P, DTILES, CHUNK], BF16, tag="x",
                                  name="xk_c")
                nc.sync.dma_start(
                    xk_c[:], pdt(xkT[:, sc * CHUNK:(sc + 1) * CHUNK]))
                psk = psP.tile([P, CHUNK], F32, tag="p", name="psk")
                for dt in range(DTILES):
                    nc.tensor.matmul(
                        psk[:],
                        wk_s[:, dt, et * P:(et + 1) * P],
                        xk_c[:, dt, :],
                        start=(dt == 0), stop=(dt == DTILES - 1))
                nc.vector.tensor_copy(
                    kt_s[:, et, sc * CHUNK:(sc + 1) * CHUNK], psk[:])

            # ---- attention building blocks ----
            ex_tiles = {}     # (qc, par, kt2) -> tile, within current hp

            def scores_slot(hp, qc, kt2):
                qsl = slice(qc * CHUNK, (qc + 1) * CHUNK)
                pst = {}
                for par in (0, 1):
                    pst[par] = psS.tile([P, 2 * CHUNK], F32, tag="s",
                                        name="pst")
                for j in (0, 1):
                    kt = 2 * kt2 + j
                    for par in (0, 1):
                        pb = DK * par
                        nc.tensor.matmul(
                            pst[par][:, j * CHUNK:(j + 1) * CHUNK],
                            kt_s[pb:pb + DK, hp, kt * P:(kt + 1) * P],
                            qt_s[pb:pb + DK, hp, qsl],
                            start=True, stop=True)
                return pst

            def exp_slot(qc, kt2, pst):
                for par in (0, 1):
                    t = epool.tile([P, 2, CHUNK], BF16, tag="e", name="ex")
                    nc.scalar.activation(t[:], pst[par][:], Exp)
                    ex_tiles[qc, par, kt2] = t

            psa_open = {}     # (qc, par) -> open accumulation psum

            def attnv_slot(hp, qc, kt2):
                # 4 matmuls: both parities, kt pair (2*kt2, 2*kt2+1)
                for par in (0, 1):
                    h = 2 * hp + par
                    key = (qc, par)
                    if kt2 == 0:
                        psa_open[key] = psA.tile([DK + 1, CHUNK], F32,
                                                 tag="a", name="psa")
                    psa = psa_open[key]
                    t = ex_tiles.pop((qc, par, kt2))
                    for j in (0, 1):
                        kt = 2 * kt2 + j
                        nc.tensor.matmul(
                            psa[:],
                            va_s[:, kt, h * (DK + 1):(h + 1) * (DK + 1)],
                            t[:, j, :],
                            start=(kt == 0), stop=(kt == KT_N - 1))

            pending = []

            def flush_normalize():
                for rsb_, par_, hp_, yt_ in pending:
                    psr = psP.tile([DK, CHUNK], F32, tag="p", name="psr")
                    nc.tensor.matmul(psr[:], ones_s[:], rsb_[:],
                                     start=True, stop=True)
                    ysl = yt_[DK * par_:DK * (par_ + 1), hp_, :]
                    nc.vector.tensor_mul(ysl, ysl, psr[:])
                pending.clear()

            def normalize(hp, qc, yt_c):
                # unnormalized copy + 1/rowsum prep; replicate+mul deferred
                for par in (0, 1):
                    psa = psa_open.pop((qc, par))
                    nc.vector.tensor_copy(
                        yt_c[DK * par:DK * (par + 1), hp, :], psa[0:DK, :])
                    rcp = npool.tile([1, CHUNK], F32, tag="rcp", name="rcp")
                    nc.vector.tensor_copy(rcp[:], psa[DK:DK + 1, :])
                    rs = npool.tile([1, CHUNK], F32, tag="rs", name="rs")
                    nc.vector.reciprocal_approx_fast(rs[:], rcp[:])
                    rsb = npool.tile([1, CHUNK], BF16, tag="rsb",
                                     name="rsb", bufs=4)
                    nc.vector.tensor_copy(rsb[:], rs[:])
                    pending.append((rsb, par, hp, yt_c))

            def outproj_tile(qc, yt_c, qtl, ec):
                psf = psP.tile([P, CHUNK], F32, tag="p", name="psf")
                for j in range(DTILES):
                    nc.tensor.matmul(
                        psf[:],
                        yt_c[:, j, qtl * P:(qtl + 1) * P],
                        wo_s[:, j, ec * CHUNK:(ec + 1) * CHUNK],
                        start=(j == 0), stop=(j == DTILES - 1))
                osb = opool.tile([P, CHUNK], F32, tag="o", name="osb")
                nc.vector.tensor_add(
                    osb[:], psf[:], bob_s[:, ec * CHUNK:(ec + 1) * CHUNK])
                r0 = qc * CHUNK + qtl * P
                nc.sync.dma_start(
                    out[r0:r0 + P, ec * CHUNK:(ec + 1) * CHUNK], osb[:])

            def outproj(qc, yt_c):
                for qtl in range(CHUNK // P):
                    for ec in range(D // CHUNK):
                        outproj_tile(qc, yt_c, qtl, ec)

            yt = {0: ytpool.tile([P, DTILES, CHUNK], BF16, tag="yt",
                                 name="yt0"),
                  1: ytpool.tile([P, DTILES, CHUNK], BF16, tag="yt",
                                 name="yt1")}

            # ---- lead-in: Kproj et 0,1 then hp0/qc0 scores+exp prefill ----
            for sc in range(S // CHUNK):
                kproj_sc(0, sc)
            for sc in range(S // CHUNK):
                kproj_sc(1, sc)
            wv_s = wpool.tile([P, DTILES, D], BF16, tag="w", name="wv_s")
            for j in range(4):
                nc.sync.dma_start(wv_s[:, 2 * j:2 * j + 2, :],
                                  pdt(wvT)[:, 2 * j:2 * j + 2, :])
            xv_c = [None]

            def vproj_st(st):
                if st % 4 == 0:
                    xv_c[0] = xpool.tile([P, DTILES, CHUNK], BF16, tag="x",
                                         name="xv_c")
                    nc.sync.dma_start(
                        xv_c[0][:],
                        pdt(xvT[:, (st // 4) * CHUNK:(st // 4 + 1) * CHUNK]))
                for ec in range(D // CHUNK):
                    psv = psP.tile([P, CHUNK], F32, tag="p", name="psv")
                    for dt in range(DTILES):
                        nc.tensor.matmul(
                            psv[:],
                            xv_c[0][:, dt, (st % 4) * P:(st % 4 + 1) * P],
                            wv_s[:, dt, ec * CHUNK:(ec + 1) * CHUNK],
                            start=(dt == 0), stop=(dt == DTILES - 1))
                    va_v = va_s.rearrange("p k (h c) -> p k h c", c=DK + 1)
                    nh = CHUNK // DK
                    nc.vector.tensor_copy(
                        va_v[:, st, ec * nh:(ec + 1) * nh, 0:DK],
                        psv.rearrange("p (h c) -> p h c", c=DK))

            # scores(0,0) prefill is ACT-paced; fill the idle PE with the
            # first 4 V-projection st-blocks
            for kt2 in range(KT_N // 2):
                pst = scores_slot(0, 0, kt2)
                exp_slot(0, kt2, pst)
                if kt2 >= 4:
                    vproj_st(kt2 - 4)

            # ---- rest of V projection, interleaved with attnV(hp0,qc0)
            #      and scores(hp0,qc1) ride-alongs ----
            attnv_slot(0, 0, 0)
            pst = scores_slot(0, 1, 0)
            exp_slot(1, 0, pst)
            pst = scores_slot(0, 1, 1)
            exp_slot(1, 1, pst)
            for st in range(4, KT_N):
                vproj_st(st)
                if st % 2 == 1:
                    attnv_slot(0, 0, (st - 3) // 2)
                    pst = scores_slot(0, 1, st // 2)
                    exp_slot(1, st // 2, pst)
            attnv_slot(0, 0, 7)
            normalize(0, 0, yt[0])

            wo_s = wpool.tile([P, DTILES, D], BF16, tag="w", name="wo_s")
            for j in range(4):
                nc.sync.dma_start(wo_s[:, 2 * j:2 * j + 2, :],
                                  pdt(woT)[:, 2 * j:2 * j + 2, :])

            # ---- steady state: block i = scores(stream i) + attnV(stream
            # i-1) + kproj(hp+1) ride-along on qc0 blocks. Stream i =
            # (hp, qc) = (i//2, i%2); streams 0 and 1 (scores) were already
            # emitted, and attnV(stream 0) ran inside the V phase. ----
            for i in range(2, 2 * HP):
                hp, qc = divmod(i, 2)
                phq, pqc = divmod(i - 1, 2)
                flush_normalize()
                for kt2 in range(KT_N // 2):
                    attnv_slot(phq, pqc, kt2)
                    pst = scores_slot(hp, qc, kt2)
                    exp_slot(qc, kt2, pst)
                    if hp + 1 < DTILES and kt2 in (2, 6):
                        kproj_sc(hp + 1, qc * 2 + kt2 // 4)
                normalize(phq, pqc, yt[pqc])

            # drain: attnV of the last stream with outproj(0) interleaved
            # (yt[0] is fully normalized by now), then outproj(1)
            flush_normalize()
            for kt2 in range(KT_N // 2):
                attnv_slot(HP - 1, 1, kt2)
                outproj_tile(0, yt[0], kt2 // 2, kt2 % 2)
            normalize(HP - 1, 1, yt[1])
            flush_normalize()
            outproj(1, yt[1])

    nc.compile()
    return nc


def _get_nc():
    if "nc" not in _CACHE:
        _CACHE["nc"] = _build_nc()
    return _CACHE["nc"]


def kernel(query, key, value, Wq, bq, Wk, bk, Wv, bv, Wo, bo):
    from concourse.bass_utils import run_bass_kernel_spmd

    bf16 = ml_dtypes.bfloat16
    query = np.asarray(query, np.float32)
    key = np.asarray(key, np.float32)
    value = np.asarray(value, np.float32)
    Wq, bq = np.asarray(Wq, np.float32), np.asarray(bq, np.float32)
    Wk = np.asarray(Wk, np.float32)
    Wv, bv = np.asarray(Wv, np.float32), np.asarray(bv, np.float32)
    Wo, bo = np.asarray(Wo, np.float32), np.asarray(bo, np.float32)

    nc = _get_nc()

    shared = {
        "wqT": np.ascontiguousarray(Wq.T).astype(bf16),
        "wkT": np.ascontiguousarray(Wk.T).astype(bf16),
        "wvT": np.ascontiguousarray(Wv.T).astype(bf16),
        "woT": np.ascontiguousarray(Wo.T).astype(bf16),
        "bqs": np.ascontiguousarray((bq / 8.0).reshape(DTILES, P).T).astype(
            np.float32),
        "bob": np.ascontiguousarray(
            np.broadcast_to(bo + Wo @ bv, (P, D))).astype(np.float32),
    }
    xkTs = [np.ascontiguousarray(key[b].T).astype(bf16) for b in range(B)]
    xvTs = [np.ascontiguousarray(value[b].T).astype(bf16) for b in range(B)]

    in_maps = []
    for c in range(NCORES):
        b, half = divmod(c, 2)
        xq = query[b, half * SQ:(half + 1) * SQ, :]
        in_maps.append({
            **shared,
            "xqT": np.ascontiguousarray(xq.T).astype(bf16),
            "xkT": xkTs[b],
            "xvT": xvTs[b],
        })

    res = run_bass_kernel_spmd(nc, in_maps, list(range(NCORES)))

    outp = np.empty((B, S, D), np.float32)
    for c in range(NCORES):
        b, half = divmod(c, 2)
        outp[b, half * SQ:(half + 1) * SQ, :] = res.results[c]["out"]
    return outp
